# revision 1
# baseline (speedup 1.0000x reference)
"""Two-layer GAT (DGL GATConv semantics) on 8 Trainium2 NeuronCores.

Self-contained kernel: kernel(**inputs) takes the FULL inputs from
reference.setup_inputs() and returns the FULL [50000, 768] output
concat([x, h1, h2]). Sharding: nodes by dst ownership (6250/core); edges
routed to dst owner; src features served from a replicated per-layer
DRAM feature table ([feat|el] rows) read with bulk dma_gather; one
AllGather of h1^T between layers; per-edge softmax/aggregation via
0/1 match-matrix matmuls accumulated in PSUM per 128-dst chunk.
"""
"""GAT 2-layer kernel for trn2 (8 NeuronCores, SPMD).

Strategy (graph-parallel, dst-ownership sharding):
- Nodes are sharded by dst ownership: core c owns nodes [c*NPC, (c+1)*NPC).
- Edges are routed to the core owning dst, sorted by dst, grouped into
  chunks of 128 dst nodes; each chunk's edges split into A (src in the low
  half of the padded id space) and B (high half) for int16 dma_gather, each
  padded to a fixed tile capacity with row-0 dummy gathers nullified by the
  match matrix (dst sentinel 128).
- Per layer a DRAM table holds rows [feat(256) | el(4) | pad] with 320-f32
  row stride (1280B, %256 for dma_gather). feat1 = x @ W1 is computed
  REPLICATED on every core (cheap, avoids an AllGather). Layer 2 exchanges
  h1^T via one AllGather, then feat2 = h1 @ W2 is recomputed replicated.
- el/er come free from the feat matmul: rhs = [W | W@al_blk | W@ar_blk].
- Aggregation + softmax denominators via TensorE: per edge-tile build a
  0/1 match matrix (dst_local == iota) and matmul-accumulate
  [msg | exp] into PSUM over the chunk; normalize once per chunk.
"""

import dataclasses
import numpy as np

import concourse.bass as bass
import concourse.bacc as bacc
import concourse.tile as tile
import concourse.mybir as mybir
from concourse.masks import make_identity

F32 = mybir.dt.float32
I16 = mybir.dt.int16
AX = mybir.AxisListType
OP = mybir.AluOpType
ACT = mybir.ActivationFunctionType

P = 128
D = 256          # feature dim (in and out)
H = 4            # heads
DH = 64          # dim per head
ROWF = 320       # table row stride in f32 (1280B, %256B)
RD = 260         # used row cols: 256 feat + 4 el
NEG_SLOPE = 0.2


@dataclasses.dataclass
class Cfg:
    N: int            # real node count
    E: int            # edge count
    NCORES: int = 8
    TA: int = 9       # gather tiles per chunk from table half A
    TB: int = 9       # gather tiles per chunk from table half B

    @property
    def NPC(self):
        return self.N // self.NCORES

    @property
    def NCHUNK(self):
        return (self.NPC + P - 1) // P

    @property
    def LASTC(self):
        return self.NPC - (self.NCHUNK - 1) * P

    @property
    def NPCPAD(self):
        return self.NCHUNK * P

    @property
    def NPAD(self):
        return self.NCORES * self.NPCPAD

    @property
    def SPLIT(self):
        assert self.NPAD % 2 == 0
        s = self.NPAD // 2
        assert s <= 32767 and self.NPAD - s <= 32767
        return s

    @property
    def T(self):
        return self.TA + self.TB

    @property
    def NGTILE(self):
        return self.NPAD // P


FULL = Cfg(N=50000, E=800000)

# profiling mode: replace the collective with local copies (single-core cost sim)
PROFILE_LOCAL_CC = False
# phase subset for ablation profiling (None = all)
PHASES = None


def _on(name):
    return PHASES is None or name in PHASES


# ---------------------------------------------------------------- host prep

def _wrap_idx(flat, ntile):
    """Flat int array [ntile*128] -> dma_gather idx sbuf layout [128, ntile*8].
    idx j lives at [j%16, j//16]; replicated across the 8 partition groups."""
    w = flat.reshape(ntile * 8, 16).T.astype(np.int16)      # [16, ntile*8]
    return np.tile(w, (8, 1))                               # [128, ntile*8]


def prep_host(cfg: Cfg, x, src, dst, W1, al1, ar1, b1, W2, al2, ar2, b2):
    """Build per-core input maps. All arrays np.float32/np.int32."""
    NPC, NCH, T, TA, TB = cfg.NPC, cfg.NCHUNK, cfg.T, cfg.TA, cfg.TB
    NPAD, SPLIT = cfg.NPAD, cfg.SPLIT

    srcp = (src // NPC) * cfg.NPCPAD + (src % NPC)          # padded ids
    core_of = dst // NPC
    loc = dst % NPC
    chunk_of = loc // P
    dloc = loc % P                                          # dst_local in chunk

    # padded transposed x table [2, 128, NPAD]
    xp = np.zeros((cfg.NCORES, cfg.NPCPAD, D), np.float32)
    xp[:, :NPC] = x.reshape(cfg.NCORES, NPC, D)
    xTf = np.ascontiguousarray(xp.reshape(NPAD, D).T.reshape(2, P, NPAD))

    def build_wrhs(W, al, ar):
        blk_l = np.zeros((D, H), np.float32)
        blk_r = np.zeros((D, H), np.float32)
        for h in range(H):
            blk_l[h * DH:(h + 1) * DH, h] = al[h]
            blk_r[h * DH:(h + 1) * DH, h] = ar[h]
        w = np.concatenate([W, W @ blk_l, W @ blk_r], axis=1)  # [256, 264]
        return np.ascontiguousarray(w.reshape(2, P, 264))

    w1rhs = build_wrhs(W1, al1, ar1)
    w2rhs = build_wrhs(W2, al2, ar2)
    b1r = np.tile(b1[None, :], (P, 1)).astype(np.float32)
    b2r = np.tile(b2[None, :], (P, 1)).astype(np.float32)
    iota_row = np.tile(np.arange(P, dtype=np.float32)[None, :], (P, 1))
    iota_col = np.arange(P, dtype=np.float32)[:, None].copy()

    in_maps = []
    for c in range(cfg.NCORES):
        m = src * 0  # placeholder
        sel = np.nonzero(core_of == c)[0]
        idxA = np.zeros((P, NCH * TA * 8), np.int16)
        idxB = np.zeros((P, NCH * TB * 8), np.int16)
        dstf = np.full((P, NCH * T), 128.0, np.float32)
        for k in range(NCH):
            ek = sel[chunk_of[sel] == k]
            sa = srcp[ek]
            dl = dloc[ek]
            a_m = sa < SPLIT
            qa, da = sa[a_m], dl[a_m]
            qb, db = sa[~a_m] - SPLIT, dl[~a_m]
            nA, nB = len(qa), len(qb)
            assert nA <= TA * P, (c, k, nA)
            assert nB <= TB * P, (c, k, nB)
            fa = np.zeros(TA * P, np.int64)
            fa[:nA] = qa
            fb = np.zeros(TB * P, np.int64)
            fb[:nB] = qb
            idxA[:, k * TA * 8:(k + 1) * TA * 8] = _wrap_idx(fa, TA)
            idxB[:, k * TB * 8:(k + 1) * TB * 8] = _wrap_idx(fb, TB)
            # slot (p, t): A edge j=(t*128+p) t<TA ; B edge j=((t-TA)*128+p)
            dcol = np.full((T, P), 128.0, np.float32)
            dcol.reshape(-1)[:nA] = da
            dcol.reshape(-1)[TA * P:TA * P + nB] = db
            dstf[:, k * T:(k + 1) * T] = dcol.T
        own = slice(c * NPC, (c + 1) * NPC)
        xTo = np.ascontiguousarray(x[own].T.reshape(2, P, NPC))
        in_maps.append({
            "xTf": xTf, "xTo": xTo,
            "idxA": idxA, "idxB": idxB, "dstf": dstf,
            "w1rhs": w1rhs, "w2rhs": w2rhs, "b1r": b1r, "b2r": b2r,
            "iota_row": iota_row, "iota_col": iota_col,
        })
    return in_maps


def assemble_output(cfg: Cfg, x, results):
    h1 = np.concatenate([r["out_h"][:, 0:D] for r in results], axis=0)
    h2 = np.concatenate([r["out_h"][:, D:2 * D] for r in results], axis=0)
    return np.concatenate([x, h1, h2], axis=1)


# ---------------------------------------------------------------- program

def build_program(cfg: Cfg):
    NPC, NCH, T, TA, TB = cfg.NPC, cfg.NCHUNK, cfg.T, cfg.TA, cfg.TB
    NPAD, SPLIT, NGT = cfg.NPAD, cfg.SPLIT, cfg.NGTILE
    LASTC = cfg.LASTC

    nc = bacc.Bacc("TRN2", target_bir_lowering=False, debug=False,
                   num_devices=cfg.NCORES, num_swdge_queues=2)

    xTf = nc.dram_tensor("xTf", [2, P, NPAD], F32, kind="ExternalInput")
    xTo = nc.dram_tensor("xTo", [2, P, NPC], F32, kind="ExternalInput")
    idxA = nc.dram_tensor("idxA", [P, NCH * TA * 8], I16, kind="ExternalInput")
    idxB = nc.dram_tensor("idxB", [P, NCH * TB * 8], I16, kind="ExternalInput")
    dstf = nc.dram_tensor("dstf", [P, NCH * T], F32, kind="ExternalInput")
    w1rhs = nc.dram_tensor("w1rhs", [2, P, 264], F32, kind="ExternalInput")
    w2rhs = nc.dram_tensor("w2rhs", [2, P, 264], F32, kind="ExternalInput")
    b1r = nc.dram_tensor("b1r", [P, D], F32, kind="ExternalInput")
    b2r = nc.dram_tensor("b2r", [P, D], F32, kind="ExternalInput")
    iota_row = nc.dram_tensor("iota_row", [P, P], F32, kind="ExternalInput")
    iota_col = nc.dram_tensor("iota_col", [P, 1], F32, kind="ExternalInput")
    out_h = nc.dram_tensor("out_h", [NPC, 2 * D], F32, kind="ExternalOutput")

    with tile.TileContext(nc) as tc:
        with tc.tile_pool(name="const", bufs=1) as cp, \
             tc.tile_pool(name="sb", bufs=3) as sb, \
             tc.tile_pool(name="sb2", bufs=2) as sb2, \
             tc.tile_pool(name="sbt", bufs=6) as sbt, \
             tc.tile_pool(name="ps", bufs=2, space="PSUM") as ps, \
             tc.tile_pool(name="ps4", bufs=4, space="PSUM") as ps4, \
             tc.tile_pool(name="dram", bufs=1, space="DRAM") as dram:

            tab1 = dram.tile([NPAD, ROWF], F32, tag="tab1")
            tab2 = dram.tile([NPAD, ROWF], F32, tag="tab2")
            h1T_own = dram.tile([2, P, cfg.NPCPAD], F32, tag="h1T_own")
            h1T_full = dram.tile([cfg.NCORES, 2, P, cfg.NPCPAD], F32,
                                 tag="h1T_full",
                                 addr_space="Local" if PROFILE_LOCAL_CC else "Shared")

            # ---- persistent SBUF ----
            w1_s = cp.tile([P, 2, 264], F32, tag="w1_s")
            w2_s = cp.tile([P, 2, 264], F32, tag="w2_s")
            b1_s = cp.tile([P, D], F32, tag="b1_s")
            b2_s = cp.tile([P, D], F32, tag="b2_s")
            irow_s = cp.tile([P, P], F32, tag="irow_s")
            icol_s = cp.tile([P, 1], F32, tag="icol_s")
            ident_s = cp.tile([P, P], F32, tag="ident_s")
            idxA_s = cp.tile([P, NCH * TA * 8], I16, tag="idxA_s")
            idxB_s = cp.tile([P, NCH * TB * 8], I16, tag="idxB_s")
            dstf_s = cp.tile([P, NCH * T], F32, tag="dstf_s")
            er1_s = cp.tile([P, NCH * H], F32, tag="er1_s")
            er2_s = cp.tile([P, NCH * H], F32, tag="er2_s")
            xTo_s = cp.tile([P, 2, NPC], F32, tag="xTo_s")

            for d in range(2):
                nc.sync.dma_start(w1_s[:, d, :], w1rhs[d])
                nc.sync.dma_start(w2_s[:, d, :], w2rhs[d])
                nc.sync.dma_start(xTo_s[:, d, :], xTo[d])
            nc.sync.dma_start(b1_s[:], b1r[:])
            nc.sync.dma_start(b2_s[:], b2r[:])
            nc.sync.dma_start(irow_s[:], iota_row[:])
            nc.sync.dma_start(icol_s[:], iota_col[:])
            nc.sync.dma_start(idxA_s[:], idxA[:])
            nc.sync.dma_start(idxB_s[:], idxB[:])
            nc.sync.dma_start(dstf_s[:], dstf[:])
            make_identity(nc, ident_s[:])
            nc.vector.memset(er1_s[:], 0.0)
            nc.vector.memset(er2_s[:], 0.0)

            def feat_phase(w_s, strip, tab, B):
                """feat+el for ALL padded nodes -> tab rows (replicated).
                B tiles per DMA batch; strip(gb, d) -> [P, B*P] dram AP."""
                for gb in range(NGT // B):
                    xs = sb.tile([P, 2, B * P], F32, tag="xsl",
                                 name=f"xsl{B}")
                    for d in range(2):
                        nc.sync.dma_start(xs[:, d, :], strip(gb, d))
                    t = sb.tile([P, B, RD], F32, tag="trow",
                                name=f"trow{B}")
                    for i in range(B):
                        f_ps = ps.tile([P, 264], F32, tag="agg")
                        for d in range(2):
                            nc.tensor.matmul(
                                out=f_ps[:], lhsT=xs[:, d, i * P:(i + 1) * P],
                                rhs=w_s[:, d, :],
                                start=(d == 0), stop=(d == 1))
                        nc.vector.tensor_copy(t[:, i, :], f_ps[:, 0:RD])
                    nc.sync.dma_start(
                        tab[gb * B * P:(gb + 1) * B * P, 0:RD].rearrange(
                            "(b p) f -> p b f", p=P),
                        t[:])

            def er_phase(w_s, xT_s, er_s):
                """er for OWN nodes in local-chunk layout."""
                for k in range(NCH):
                    rows = LASTC if k == NCH - 1 else P
                    e_ps = ps.tile([P, H], F32, tag="ere")
                    for d in range(2):
                        nc.tensor.matmul(
                            out=e_ps[:rows],
                            lhsT=xT_s[:, d, k * P:k * P + rows],
                            rhs=w_s[:, d, 260:264],
                            start=(d == 0), stop=(d == 1))
                    nc.vector.tensor_copy(er_s[:rows, k * H:(k + 1) * H],
                                          e_ps[:rows])

            def edge_phase(tab, er_s, b_s, layer):
                gbuf = [cp.tile([P, T * ROWF], F32, tag=f"G{i}",
                                name=f"G{i}_l{layer}")
                        for i in range(2)]
                for k in range(NCH):
                    rows = LASTC if k == NCH - 1 else P
                    G = gbuf[k % 2]
                    GA = G[:, 0:TA * ROWF].rearrange("p (t f) -> p t f", f=ROWF)
                    GB = G[:, TA * ROWF:T * ROWF].rearrange(
                        "p (t f) -> p t f", f=ROWF)
                    nc.gpsimd.dma_gather(
                        GA, tab[0:SPLIT, :],
                        idxA_s[:, k * TA * 8:(k + 1) * TA * 8],
                        TA * P, TA * P, ROWF, elem_step=ROWF, queue_num=0,
                        single_packet=False)
                    nc.gpsimd.dma_gather(
                        GB, tab[SPLIT:NPAD, :],
                        idxB_s[:, k * TB * 8:(k + 1) * TB * 8],
                        TB * P, TB * P, ROWF, elem_step=ROWF, queue_num=1,
                        single_packet=False)

                    erc = er_s[:, k * H:(k + 1) * H]
                    # pass 1: er expansion for every edge slot
                    ere_ps = ps.tile([P, T * H], F32, tag="ere")
                    for t in range(T):
                        dcol = dstf_s[:, k * T + t:k * T + t + 1]
                        dT_ps = ps4.tile([P, P], F32, tag="dT")
                        nc.tensor.transpose(out=dT_ps[:],
                                            in_=dcol.to_broadcast([P, P]),
                                            identity=ident_s[:])
                        mT = sbt.tile([P, P], F32, tag="mT")
                        nc.vector.tensor_tensor(
                            out=mT[:], in0=icol_s[:].to_broadcast([P, P]),
                            in1=dT_ps[:], op=OP.is_equal)
                        nc.tensor.matmul(out=ere_ps[:, t * H:(t + 1) * H],
                                         lhsT=mT[:], rhs=erc,
                                         start=True, stop=True)
                    # s = el + er ; ex = exp(leaky_relu(s))
                    exa = sb2.tile([P, T * H], F32, tag="exa")
                    nc.vector.tensor_tensor(
                        out=exa[:].rearrange("p (t f) -> p t f", f=H),
                        in0=G[:].rearrange("p (t f) -> p t f", f=ROWF)[:, :, 256:260],
                        in1=ere_ps[:].rearrange("p (t f) -> p t f", f=H),
                        op=OP.add)
                    lrt = sb2.tile([P, T * H], F32, tag="lrt")
                    nc.vector.tensor_scalar_mul(lrt[:], exa[:], NEG_SLOPE)
                    nc.vector.tensor_tensor(out=exa[:], in0=exa[:], in1=lrt[:],
                                            op=OP.max)
                    nc.scalar.activation(out=exa[:], in_=exa[:], func=ACT.Exp)
                    # pass 2: weighted messages + aggregation matmuls
                    agg_ps = ps.tile([P, D], F32, tag="agg")
                    exagg_ps = ps.tile([P, H], F32, tag="ere", name="exagg")
                    for t in range(T):
                        dcol = dstf_s[:, k * T + t:k * T + t + 1]
                        m = sbt.tile([P, P], F32, tag="m")
                        nc.vector.tensor_tensor(
                            out=m[:], in0=dcol.to_broadcast([P, P]),
                            in1=irow_s[:], op=OP.is_equal)
                        C = sbt.tile([P, D], F32, tag="C")
                        nc.vector.tensor_tensor(
                            out=C[:].rearrange("p (h d) -> p h d", h=H),
                            in0=G[:, t * ROWF:t * ROWF + D].rearrange(
                                "p (h d) -> p h d", h=H),
                            in1=exa[:, t * H:(t + 1) * H, None].to_broadcast(
                                [P, H, DH]),
                            op=OP.mult)
                        nc.tensor.matmul(out=agg_ps[:], lhsT=m[:],
                                         rhs=C[:],
                                         start=(t == 0), stop=(t == T - 1))
                        nc.tensor.matmul(out=exagg_ps[:], lhsT=m[:],
                                         rhs=exa[:, t * H:(t + 1) * H],
                                         start=(t == 0), stop=(t == T - 1))
                    # finalize: normalize + bias (+elu, feat2 on layer 1)
                    den = sb.tile([P, H], F32, tag="den")
                    nc.vector.tensor_scalar(out=den[:], in0=exagg_ps[:],
                                            scalar1=1e-30, scalar2=None,
                                            op0=OP.max)
                    rden = sb.tile([P, H], F32, tag="rden")
                    nc.vector.reciprocal(rden[:], den[:])
                    hmat = sb.tile([P, D], F32, tag="hmat")
                    nc.vector.tensor_tensor(
                        out=hmat[:].rearrange("p (h d) -> p h d", h=H),
                        in0=agg_ps[:].rearrange("p (h d) -> p h d", h=H),
                        in1=rden[:, :, None].to_broadcast([P, H, DH]),
                        op=OP.mult)
                    nc.vector.tensor_tensor(out=hmat[:], in0=hmat[:],
                                            in1=b_s[:], op=OP.add)
                    if layer == 1:
                        tmin = sb.tile([P, D], F32, tag="tmin")
                        nc.vector.tensor_scalar_min(tmin[:], hmat[:], 0.0)
                        nc.scalar.activation(out=tmin[:], in_=tmin[:],
                                             func=ACT.Exp)
                        nc.vector.tensor_scalar_add(tmin[:], tmin[:], -1.0)
                        nc.vector.tensor_tensor(out=hmat[:], in0=hmat[:],
                                                in1=tmin[:], op=OP.max)
                        nc.sync.dma_start(out_h[k * P:k * P + rows, 0:D],
                                          hmat[:rows])
                        e2_ps = ps.tile([P, H], F32, tag="ere")
                        for d in range(2):
                            tr_ps = ps4.tile([P, P], F32, tag="dT")
                            nc.tensor.transpose(
                                out=tr_ps[:], in_=hmat[:, d * P:(d + 1) * P],
                                identity=ident_s[:])
                            hT = sb.tile([P, P], F32, tag="hT")
                            nc.vector.tensor_copy(hT[:], tr_ps[:])
                            nc.sync.dma_start(
                                h1T_own[d, :, k * P:(k + 1) * P], hT[:])
                            nc.tensor.matmul(
                                out=e2_ps[:rows], lhsT=hT[:, :rows],
                                rhs=w2_s[:, d, 260:264],
                                start=(d == 0), stop=(d == 1))
                        nc.vector.tensor_copy(
                            er2_s[:rows, k * H:(k + 1) * H], e2_ps[:rows])
                    else:
                        nc.sync.dma_start(out_h[k * P:k * P + rows, D:2 * D],
                                          hmat[:rows])

            # ---------------- layer 1 ----------------
            if _on("feat1"):
                feat_phase(w1_s,
                           lambda gb, d: xTf[d, :, gb * 8 * P:(gb + 1) * 8 * P],
                           tab1, B=8)
            if _on("er1"):
                er_phase(w1_s, xTo_s, er1_s)
            if _on("edge1"):
                edge_phase(tab1, er1_s, b1_s, layer=1)

            # ---------------- exchange h1 ----------------
            if not _on("cc"):
                pass
            elif PROFILE_LOCAL_CC:
                for c in range(cfg.NCORES):
                    nc.gpsimd.dma_start(h1T_full[c], h1T_own[:])
            else:
                nc.gpsimd.collective_compute(
                    "AllGather", OP.bypass,
                    replica_groups=[list(range(cfg.NCORES))],
                    ins=[h1T_own.opt()], outs=[h1T_full.opt()])

            # ---------------- layer 2 ----------------
            # feat2 for all nodes from gathered h1T
            assert NCH % 7 == 0 or NCH < 7
            B2 = 7 if NCH % 7 == 0 else 1
            def h1t_strip(gb, d):
                c, kb = divmod(gb, NCH // B2)
                return h1T_full[c, d, :, kb * B2 * P:(kb + 1) * B2 * P]
            if _on("feat2"):
                feat_phase(w2_s, h1t_strip, tab2, B=B2)
            if _on("edge2"):
                edge_phase(tab2, er2_s, b2_s, layer=2)

    nc.compile()
    return nc


# ------------------------------------------------------------ numpy reference

def ref_numpy(cfg: Cfg, x, src, dst, W1, al1, ar1, b1, W2, al2, ar2, b2):
    def gat(x, W, al, ar, b, elu):
        feat = (x @ W).reshape(cfg.N, H, DH)
        el = np.einsum("nhd,hd->nh", feat, al)
        er = np.einsum("nhd,hd->nh", feat, ar)
        e = el[src] + er[dst]
        e = np.where(e > 0, e, NEG_SLOPE * e)
        ex = np.exp(e)
        denom = np.zeros((cfg.N, H), np.float32)
        np.add.at(denom, dst, ex)
        out = np.zeros((cfg.N, H, DH), np.float32)
        np.add.at(out, dst, feat[src] * (ex / np.maximum(denom[dst], 1e-30))[..., None])
        out = out + b.reshape(1, H, DH)
        if elu:
            out = np.where(out > 0, out, np.exp(np.minimum(out, 0)) - 1)
        return out.reshape(cfg.N, D).astype(np.float32)

    h1 = gat(x, W1, al1, ar1, b1, elu=True)
    h2 = gat(h1, W2, al2, ar2, b2, elu=False)
    return np.concatenate([x, h1, h2], axis=1)


def make_tiny_inputs(cfg: Cfg, seed=0):
    rng = np.random.default_rng(seed)
    x = rng.standard_normal((cfg.N, D), dtype=np.float32)
    src = rng.integers(0, cfg.N, cfg.E).astype(np.int32)
    dst = rng.integers(0, cfg.N, cfg.E).astype(np.int32)
    s1 = 1.0 / np.sqrt(D)
    W1 = (rng.standard_normal((D, D), dtype=np.float32) * s1)
    al1 = (rng.standard_normal((H, DH), dtype=np.float32) * s1)
    ar1 = (rng.standard_normal((H, DH), dtype=np.float32) * s1)
    b1 = np.zeros(D, np.float32)
    W2 = (rng.standard_normal((D, D), dtype=np.float32) * s1)
    al2 = (rng.standard_normal((H, DH), dtype=np.float32) * s1)
    ar2 = (rng.standard_normal((H, DH), dtype=np.float32) * s1)
    b2 = np.zeros(D, np.float32)
    return dict(x=x, src=src, dst=dst, W1=W1, al1=al1, ar1=ar1, b1=b1,
                W2=W2, al2=al2, ar2=ar2, b2=b2)


# ----------------------------- PJRT SPMD runner
import jax
import jax.numpy as jnp
from jax.experimental.shard_map import shard_map
from jax.sharding import Mesh, PartitionSpec
from concourse.bass2jax import _bass_exec_p, install_neuronx_cc_hook, partition_id_tensor

import numpy as np
import jax
import jax.numpy as jnp
from jax.experimental.shard_map import shard_map
from jax.sharding import Mesh, PartitionSpec

import concourse.mybir as mybir
from concourse import bass2jax
from concourse.bass2jax import _bass_exec_p, install_neuronx_cc_hook, partition_id_tensor


class SpmdRunner:
    def __init__(self, nc, n_cores):
        install_neuronx_cc_hook()
        self.nc = nc
        self.n_cores = n_cores
        partition_name = (nc.partition_id_tensor.name
                          if nc.partition_id_tensor else None)
        in_names, out_names, out_avals, zero_outs = [], [], [], []
        for alloc in nc.m.functions[0].allocations:
            if not isinstance(alloc, mybir.MemoryLocationSet):
                continue
            name = alloc.memorylocations[0].name
            if alloc.kind == "ExternalInput":
                if name != partition_name:
                    in_names.append(name)
            elif alloc.kind == "ExternalOutput":
                shape = tuple(alloc.tensor_shape)
                dtype = mybir.dt.np(alloc.dtype)
                out_names.append(name)
                out_avals.append(jax.core.ShapedArray(shape, dtype))
                zero_outs.append(np.zeros(shape, dtype))
        self.in_names, self.out_names = in_names, out_names
        self.zero_outs = zero_outs
        n_params = len(in_names)
        n_outs = len(out_avals)
        all_names = list(in_names) + list(out_names)
        if partition_name is not None:
            all_names.append(partition_name)

        def _body(*args):
            operands = list(args)
            if partition_name is not None:
                operands.append(partition_id_tensor())
            outs = _bass_exec_p.bind(
                *operands,
                out_avals=tuple(out_avals),
                in_names=tuple(all_names),
                out_names=tuple(out_names),
                lowering_input_output_aliases=(),
                sim_require_finite=False,
                sim_require_nnan=False,
                nc=nc,
            )
            return tuple(outs)

        devices = jax.devices()[:n_cores]
        self.mesh = Mesh(np.asarray(devices), ("core",))
        in_specs = (PartitionSpec("core"),) * (n_params + n_outs)
        out_specs = (PartitionSpec("core"),) * n_outs
        donate = tuple(range(n_params, n_params + n_outs))
        self.sharded = jax.jit(
            shard_map(_body, mesh=self.mesh, in_specs=in_specs,
                      out_specs=out_specs, check_rep=False),
            donate_argnums=donate, keep_unused=True)
        self.n_params = n_params
        self.staged = None

    def stage(self, in_maps):
        """Concat per-core inputs and move to devices once."""
        concat = [np.concatenate([np.asarray(in_maps[c][n])
                                  for c in range(self.n_cores)], axis=0)
                  for n in self.in_names]
        sharding = jax.sharding.NamedSharding(self.mesh, PartitionSpec("core"))
        self.staged = [jax.device_put(a, sharding) for a in concat]
        zshapes = [((self.n_cores * z.shape[0],) + z.shape[1:], z.dtype)
                   for z in self.zero_outs]
        self.zero_fn = jax.jit(
            lambda: tuple(jnp.zeros(s, d) for s, d in zshapes),
            out_shardings=tuple(sharding for _ in zshapes))

    def run(self):
        zeros = self.zero_fn()
        jax.block_until_ready(zeros)
        out_arrs = self.sharded(*self.staged, *zeros)
        jax.block_until_ready(out_arrs)
        return out_arrs

    def results(self, out_arrs):
        res = []
        for c in range(self.n_cores):
            d = {}
            for i, name in enumerate(self.out_names):
                full = np.asarray(out_arrs[i])
                per = full.reshape(self.n_cores, -1, *full.shape[1:])[c]
                d[name] = per
            res.append(d)
        return res


# ----------------------------- public entry point

_CACHE = {}


def kernel(x, src, dst, W1, al1, ar1, b1, W2, al2, ar2, b2):
    cfg = FULL
    x = np.asarray(x, np.float32)
    src = np.asarray(src, np.int32)
    dst = np.asarray(dst, np.int32)
    args = [np.asarray(a, np.float32) for a in
            (W1, al1, ar1, b1, W2, al2, ar2, b2)]
    in_maps = prep_host(cfg, x, src, dst, *args)
    if "runner" not in _CACHE:
        nc = build_program(cfg)
        _CACHE["runner"] = SpmdRunner(nc, cfg.NCORES)
    r = _CACHE["runner"]
    r.stage(in_maps)
    out = r.run()
    res = r.results(out)
    return assemble_output(cfg, x, res)

